# revision 1
# baseline (speedup 1.0000x reference)
"""NeuMissBlock Trainium2 kernel.

h_{t+1} = obs * (h_t @ W.T) + h0, depth steps, obs = ~isnan(x),
h0 = obs*(x - mu). Data-parallel over 8 NeuronCores (4096 rows each).

Per-core plan (feature-major): keep h transposed as hT [512f, 512b] tiles so
the PE contraction runs over features. Matmul operands are float32r
(tf32-like: ~1e-4 rel err at bf16-rate). Per step and output f-tile j:

  variant "a":  psum = I.T @ h0T_j (start=True) + sum_kt wT(kt,j).T @ hT_kt
                hT'_j = psum * obsT_j                     (DVE)
  variant "b":  psum = sum_kt wT(kt,j).T @ hT_kt
                g_j  = psum + h0T_j                       (DVE)
                hT'_j = g_j * obsT_j                      (Pool/GpSimd)

Boundary transposes (x in, h out, W once) via PE transpose + ACT evac copies.
B-tiles are processed in interleaved pairs so the in-order PE queue always has
independent work while DVE/Pool finish a step.
"""
import numpy as np

BATCH = 32768
F = 512
N_CORES = 8
ROWS = BATCH // N_CORES   # 4096
BT = 512                  # batch rows per b-tile
P = 128
NF = F // P               # 4 f-tiles / k-tiles

BEST_VARIANT = "d"
CSPLIT = 4  # variant c: every CSPLIT-th tile uses a-style (identity MM + DVE mult)

_cache: dict = {}


def _build(rows: int, depth: int, variant: str = BEST_VARIANT):
    import concourse.tile as tile
    from concourse import bacc, mybir
    from concourse.masks import make_identity

    f32 = mybir.dt.float32
    f32r = mybir.dt.float32r
    i32 = mybir.dt.int32
    nbt = rows // BT
    assert rows % BT == 0

    nc = bacc.Bacc("TRN2", target_bir_lowering=False, debug=False,
                   num_devices=N_CORES)
    x_ap = nc.dram_tensor("x", [rows, F], f32, kind="ExternalInput").ap()
    mu_ap = nc.dram_tensor("mu", [F], f32, kind="ExternalInput").ap()
    w_ap = nc.dram_tensor("W", [F, F], f32, kind="ExternalInput").ap()
    out_ap = nc.dram_tensor("out", [rows, F], f32, kind="ExternalOutput").ap()

    if variant == "noop":
        with tile.TileContext(nc) as tc:
            with tc.tile_pool(name="sbuf", bufs=2) as pool:
                for bt in range(nbt):
                    for i in range(NF):
                        t = pool.tile([P, F], f32, tag="t")
                        nc.sync.dma_start(
                            t[:], x_ap[bt * BT + i * P: bt * BT + (i + 1) * P, :])
                        nc.sync.dma_start(
                            out_ap[bt * BT + i * P: bt * BT + (i + 1) * P, :], t[:])
        nc.compile()
        return nc

    with tile.TileContext(nc) as tc:
        with (
            tc.tile_pool(name="const", bufs=1) as cpool,
            tc.tile_pool(name="work", bufs=1) as wpool,
            tc.tile_pool(name="io", bufs=2) as iopool,
            tc.tile_pool(name="iout", bufs=(1 if variant in ("b", "c", "d", "e") else 2)) as opool,
            tc.tile_pool(name="psum", bufs=(2 if variant == "e" else 8),
                         space="PSUM") as pspool,
            tc.tile_pool(name="psum3", bufs=2, space="PSUM") as pspool3,
        ):
            # ---- constants ----
            ident = cpool.tile([P, P], f32, tag="ident")
            make_identity(nc, ident[:])
            if variant in ("a", "c", "d", "e"):
                ident_r = cpool.tile([P, P], f32r, tag="ident_r")
                nc.vector.tensor_copy(ident_r[:], ident[:])

            mu_sb = cpool.tile([P, NF], f32, tag="mu")
            nc.sync.dma_start(mu_sb[:], mu_ap.rearrange("(t p) -> p t", p=P))
            negmu = cpool.tile([P, NF], f32, tag="negmu")
            nc.vector.tensor_scalar_mul(negmu[:], mu_sb[:], -1.0)

            # ---- W: load batch-major, transpose to lhsT tiles (f32r) ----
            wB = cpool.tile([P, NF * F], f32, tag="wB")  # ft-th block: W rows
            for ft in range(NF):
                nc.sync.dma_start(wB[:, ft * F:(ft + 1) * F],
                                  w_ap[ft * P:(ft + 1) * P, :])
            wT = cpool.tile([P, NF * F], f32r, tag="wT")

            def w_transposes():
                for kt in range(NF):
                    ps = pspool.tile([P, F], f32, tag="ps")
                    for ft in range(NF):
                        nc.tensor.transpose(
                            ps[:, ft * P:(ft + 1) * P],
                            wB[:, ft * F + kt * P: ft * F + (kt + 1) * P],
                            ident[:])
                    nc.scalar.copy(wT[:, kt * F:(kt + 1) * F], ps[:])

            if variant not in ("d", "e"):
                w_transposes()

            def lhsT(kt, ft):
                return wT[:, kt * F + ft * P: kt * F + (ft + 1) * P]

            # ---- per-b-tile stages ----
            def load(bt, half):
                xB = iopool.tile([P, NF * F], f32, tag=f"xB{half}")
                for i in range(NF):
                    nc.sync.dma_start(
                        xB[:, i * F:(i + 1) * F],
                        x_ap[bt * BT + i * P: bt * BT + (i + 1) * P, :])
                return xB

            def setup(bt, half, xB=None):
                if xB is None:
                    xB = load(bt, half)
                xT = wpool.tile([P, NF * BT], f32, tag=f"xT{half}")
                obs = wpool.tile([P, NF * BT], f32, tag=f"obs{half}")
                cthis = wpool.tile([P, NF * BT], f32, tag=f"c{half}")
                h0f = wpool.tile([P, NF * BT], f32, tag=f"h0f{half}")
                nc.gpsimd.memset(h0f[:], 0.0)
                st = dict(obs=obs)
                if variant == "a":
                    h0 = wpool.tile([P, NF * BT], f32r, tag=f"h0{half}")
                    st["h0"] = h0
                else:
                    st["h0"] = h0f
                if variant in ("c", "d", "e"):
                    h0r = wpool.tile([P, NF * BT], f32r, tag=f"h0r{half}")
                    st["h0r"] = h0r
                for j in range(NF):
                    jj = slice(j * BT, (j + 1) * BT)
                    ps = pspool.tile([P, BT], f32, tag="ps")
                    for i in range(NF):
                        nc.tensor.transpose(
                            ps[:, i * P:(i + 1) * P],
                            xB[:, i * F + j * P: i * F + (j + 1) * P],
                            ident[:])
                    nc.scalar.copy(xT[:, jj], ps[:])
                    nc.vector.tensor_tensor(obs[:, jj], xT[:, jj], xT[:, jj],
                                            mybir.AluOpType.is_equal)
                    nc.scalar.activation(cthis[:, jj], xT[:, jj],
                                         mybir.ActivationFunctionType.Identity,
                                         bias=negmu[:, j:j + 1])
                    nc.vector.copy_predicated(
                        h0f[:, jj], obs[:, jj].bitcast(i32), cthis[:, jj])
                    if variant == "a":
                        nc.scalar.copy(st["h0"][:, jj], h0f[:, jj])
                    elif variant in ("c", "d", "e"):
                        nc.scalar.copy(st["h0r"][:, jj], h0f[:, jj])
                hA = wpool.tile([P, NF * BT], f32r, tag=f"hA{half}")
                hB = wpool.tile([P, NF * BT], f32r, tag=f"hB{half}")
                hOut = wpool.tile([P, NF * BT], f32, tag=f"hOut{half}")
                st.update(hA=hA, hB=hB, hOut=hOut)
                if variant == "b":
                    # first step's rhs must be f32r: cast h0f once
                    h0r = wpool.tile([P, NF * BT], f32r, tag=f"h0r{half}")
                    nc.scalar.copy(h0r[:], h0f[:])
                    st["h0r"] = h0r
                if variant in ("b", "c", "d", "e"):
                    g = wpool.tile([P, NF * BT], f32, tag=f"g{half}")
                    st["g"] = g
                return st

            cidx = [0]

            def step(st, t):
                h0, obs = st["h0"], st["obs"]
                if variant == "a":
                    src = h0 if t == 0 else (
                        st["hA"] if t % 2 == 1 else st["hB"])
                else:
                    src = st["h0r"] if t == 0 else (
                        st["hA"] if t % 2 == 1 else st["hB"])
                dst = st["hOut"] if t == depth - 1 else (
                    st["hA"] if t % 2 == 0 else st["hB"])
                if variant == "e":
                    # 3 b-style groups share one 3-bank psum tile; j=3 a-style
                    g = st["g"]
                    ps3 = pspool3.tile([P, 3 * BT], f32, tag="ps3")
                    for j in range(3):
                        jj = slice(j * BT, (j + 1) * BT)
                        for kt in range(NF):
                            nc.tensor.matmul(
                                ps3[:, jj], lhsT(kt, j),
                                src[:, kt * BT:(kt + 1) * BT],
                                start=(kt == 0), stop=(kt == NF - 1))
                    j3 = slice(3 * BT, 4 * BT)
                    psA = pspool.tile([P, BT], f32, tag="ps")
                    nc.tensor.matmul(psA[:], ident_r[:], st["h0r"][:, j3],
                                     start=True, stop=False)
                    for kt in range(NF):
                        nc.tensor.matmul(
                            psA[:], lhsT(kt, 3),
                            src[:, kt * BT:(kt + 1) * BT],
                            start=False, stop=(kt == NF - 1))
                    j03 = slice(0, 3 * BT)
                    nc.vector.tensor_tensor(g[:, j03], ps3[:], h0[:, j03],
                                            mybir.AluOpType.add)
                    nc.vector.tensor_tensor(dst[:, j3], psA[:], obs[:, j3],
                                            mybir.AluOpType.mult)
                    nc.gpsimd.tensor_tensor(dst[:, j03], g[:, j03],
                                            obs[:, j03],
                                            mybir.AluOpType.mult)
                    return
                for j in range(NF):
                    jj = slice(j * BT, (j + 1) * BT)
                    ps = pspool.tile([P, BT], f32, tag="ps")
                    if variant in ("c", "d"):
                        cidx[0] += 1
                        style = "a" if cidx[0] % CSPLIT == 0 else "b"
                    else:
                        style = variant
                    if style == "a":
                        id_rhs = h0 if variant == "a" else st["h0r"]
                        nc.tensor.matmul(ps[:], ident_r[:], id_rhs[:, jj],
                                         start=True, stop=False)
                        for kt in range(NF):
                            nc.tensor.matmul(
                                ps[:], lhsT(kt, j),
                                src[:, kt * BT:(kt + 1) * BT],
                                start=False, stop=(kt == NF - 1))
                        nc.vector.tensor_tensor(dst[:, jj], ps[:], obs[:, jj],
                                                mybir.AluOpType.mult)
                    else:
                        for kt in range(NF):
                            nc.tensor.matmul(
                                ps[:], lhsT(kt, j),
                                src[:, kt * BT:(kt + 1) * BT],
                                start=(kt == 0), stop=(kt == NF - 1))
                        g = st["g"]
                        nc.vector.tensor_tensor(g[:, jj], ps[:], h0[:, jj],
                                                mybir.AluOpType.add)
                        nc.gpsimd.tensor_tensor(dst[:, jj], g[:, jj],
                                                obs[:, jj],
                                                mybir.AluOpType.mult)

            def teardown(bt, st):
                hOut = st["hOut"]
                outB = opool.tile([P, NF * F], f32, tag="outB")
                for i in range(NF):
                    ii = slice(i * F, (i + 1) * F)
                    ps = pspool.tile([P, F], f32, tag="ps")
                    for j in range(NF):
                        nc.tensor.transpose(
                            ps[:, j * P:(j + 1) * P],
                            hOut[:, j * BT + i * P: j * BT + (i + 1) * P],
                            ident[:])
                    if variant in ("d", "e"):
                        nc.vector.tensor_copy(outB[:, ii], ps[:])
                    else:
                        nc.scalar.copy(outB[:, ii], ps[:])
                    nc.sync.dma_start(
                        out_ap[bt * BT + i * P: bt * BT + (i + 1) * P, :],
                        outB[:, ii])

            assert nbt % 2 == 0
            npairs = nbt // 2
            if variant in ("d", "e"):
                xBs = [load(0, 0), load(1, 1)]
                sts = [setup(0, 0, xBs[0]), setup(1, 1, xBs[1])]
                w_transposes()
                for pr in range(npairs):
                    bts = (2 * pr, 2 * pr + 1)
                    nxt = (2 * pr + 2, 2 * pr + 3)
                    for t in range(depth):
                        for h in (0, 1):
                            step(sts[h], t)
                        if t == 0 and pr + 1 < npairs:
                            xBs = [load(nxt[0], 0), load(nxt[1], 1)]
                    if pr + 1 < npairs:
                        sts_next = [setup(nxt[0], 0, xBs[0]),
                                    setup(nxt[1], 1, xBs[1])]
                    for h in (0, 1):
                        teardown(bts[h], sts[h])
                    if pr + 1 < npairs:
                        sts = sts_next
            else:
                for pr in range(npairs):
                    bts = (2 * pr, 2 * pr + 1)
                    sts = [setup(bts[h], h) for h in (0, 1)]
                    for t in range(depth):
                        for h in (0, 1):
                            step(sts[h], t)
                    for h in (0, 1):
                        teardown(bts[h], sts[h])

    nc.compile()
    return nc


def _get(rows, depth):
    key = (rows, depth)
    if key not in _cache:
        _cache[key] = _build(rows, depth)
    return _cache[key]


def kernel(x, mu, W, depth):
    from concourse.bass_utils import run_bass_kernel_spmd

    depth = int(depth)
    x = np.ascontiguousarray(x, dtype=np.float32)
    mu = np.ascontiguousarray(mu, dtype=np.float32)
    W = np.ascontiguousarray(W, dtype=np.float32)
    if depth < 1:
        miss = np.isnan(x)
        obs = (~miss).astype(np.float32)
        return np.where(miss, 0.0, x) - obs * mu
    nc = _get(x.shape[0] // N_CORES, depth)
    shards = np.split(x, N_CORES, axis=0)
    in_maps = [{"x": s, "mu": mu, "W": W} for s in shards]
    res = run_bass_kernel_spmd(nc, in_maps, core_ids=list(range(N_CORES)))
    return np.concatenate([res.results[i]["out"] for i in range(N_CORES)],
                          axis=0)

